# revision 30
# baseline (speedup 1.0000x reference)
"""Trainium2 Bass kernel for nn_DecoderLayer_84404697301735.

3-sublayer decoder (self-attn w/ char rel-pos, cross-attn to char encoder
w/ rel-pos, cross-attn to word encoder w/ word-level pos) + FFN.

Sharding: 8 cores = 4 batch x 2 interleaved query-tile halves.  Each core
computes 512 query rows end-to-end (feature-major layout); K/V projections
over the full 1024 keys are duplicated within a batch pair.  No collectives.

v2 (perf rewrite over the v1 baseline):
- P^T for the PV matmul produced by TensorE transposes into PSUM (+ ve/sc
  copies) instead of DMA-transposes, which serialized on the Scalar engine
  and saturated a DMA ring.
- Weight loads batched: one DMA per 128-column block loading all 8/32
  contraction tiles ([128, nkt*128] contiguous, 2KB descriptors).
- V projection computed key-major directly (enc tiles as lhsT, natural-
  layout Wv as rhs) - no PE transposes of V, V bias folded into O bias
  host-side (softmax rows sum to 1).
- Band table narrowed 1152 -> 896; causal mask applied via a broadcast
  mask tile added during the PSUM->SBUF band copy (no ones-row in Q).
- Q^T and K^T stored as head-pairs on 128 partitions (halves SBUF).
- Sublayer-1 skips statically-masked key blocks (kt > 2l+1).
- PV accumulates per head over key tiles with N=512 rhs.
"""

import numpy as np
import ml_dtypes

import concourse.bass as bass
import concourse.tile as tile
from concourse import bacc, mybir
from concourse.bass_utils import run_bass_kernel_spmd

BF16 = ml_dtypes.bfloat16
F32 = mybir.dt.float32
BF = mybir.dt.bfloat16

D = 1024
H = 16
DH = 64
S_OWN = 512          # own query rows per core
LK = 1024            # keys
DFF = 4096
M = 128              # pos clip radius
W = 896              # band table width
TB = 384             # band base offset (t0 = 128*(kt-2l) + TB)
T_LO, T_HI = 128, 768
SCALE = float(DH) ** 0.5   # 8.0
EPS = 1e-5
NEG = -1e30

AL = mybir.AluOpType
AF = mybir.ActivationFunctionType

# bias_cat column layout (each unit = 1 col of [128, n] per-partition stripes)
_BIAS_SECTS = [
    ("q1", 8), ("k1", 8), ("o1", 8), ("q2", 8), ("k2", 8), ("o2", 8),
    ("q3", 8), ("k3", 8), ("o3", 8), ("f1", 32), ("f2", 8),
    ("ln1g", 8), ("ln1b", 8), ("ln2g", 8), ("ln2b", 8), ("ln3g", 8), ("ln3b", 8),
]
BIAS_COL = {}
_c = 0
for _n, _w in _BIAS_SECTS:
    BIAS_COL[_n] = _c
    _c += _w
NBIAS = _c  # 152


def _t0(l, kt):
    return min(max(128 * (kt - 2 * l) + TB, T_LO), T_HI)


def _skew_segments(l, kts):
    """Plan skew DMAs for one <=4-kt chunk: list of (dest_col, n_ktiles,
    t0, affine)."""
    segs = []
    i = 0
    while i < len(kts):
        raw = 128 * (kts[i] - 2 * l) + TB
        t0 = _t0(l, kts[i])
        clamped = raw != t0
        j = i + 1
        if not clamped:
            while j < len(kts):
                if 128 * (kts[j] - 2 * l) + TB != _t0(l, kts[j]):
                    break
                j += 1
            segs.append((i * 128, j - i, t0, True))
        else:
            while j < len(kts):
                if (128 * (kts[j] - 2 * l) + TB == _t0(l, kts[j])
                        or _t0(l, kts[j]) != t0):
                    break
                j += 1
            segs.append((i * 128, j - i, t0, False))
        i = j
    return segs


def _sub1_kts(l):
    return list(range(0, min(2 * l + 2, 8)))


# columns of the band table each (causal, l) actually needs, as <=512-wide
# matmul segments (validated against the skew-read pattern in numpy)
BAND_RANGES = {
    (True, 0): [(256, 384)],
    (True, 1): [(0, 512), (512, 128)],
    (True, 2): [(0, 512), (512, 128)],
    (True, 3): [(0, 512), (512, 128)],
    (False, 0): [(256, 512), (768, 128)],
    (False, 1): [(0, 512), (512, 384)],
    (False, 2): [(0, 512), (512, 384)],
    (False, 3): [(0, 512), (512, 128)],
}


def _chunk_type(l, kts, causal):
    """strip / blo / bhi: fully-clipped chunks use a rank-1 bias instead of
    the band round trip (Q . pos[-M] resp. pos[+M], constant per row)."""
    raws = [128 * (kt - 2 * l) + TB for kt in kts]
    if all(r < T_LO for r in raws):
        return "blo"
    if (not causal) and all(r > T_HI for r in raws):
        return "bhi"
    return "strip"


def _emit(nc, tc, ctx, T, debug=False):
    """Emit the whole per-core program.  T: dict name -> dram AP."""
    te, ve, sc, gp, sy = nc.tensor, nc.vector, nc.scalar, nc.gpsimd, nc.sync

    singles = ctx.enter_context(tc.tile_pool(name="singles", bufs=1))
    psum = ctx.enter_context(tc.tile_pool(name="psum", bufs=1, space="PSUM"))
    wpool = ctx.enter_context(tc.tile_pool(name="wpool", bufs=2))
    work = ctx.enter_context(tc.tile_pool(name="work", bufs=3))
    ppool = ctx.enter_context(tc.tile_pool(name="ppool", bufs=3))
    dram = ctx.enter_context(tc.tile_pool(name="dramp", bufs=4, space="DRAM"))
    smalls = ctx.enter_context(tc.tile_pool(name="smalls", bufs=4))
    # PSUM budget (8 banks, bank-granular): mm=4, tp=2, pv=2

    # ---- persistent SBUF ----
    bias_sb = singles.tile([128, NBIAS], F32)
    sy.dma_start(bias_sb, T["bias"])
    xbf = singles.tile([128, 8, S_OWN], BF)        # bf16 copy for proj rhs
    sy.dma_start(xbf, T["xob"].rearrange("(a p) r -> p a r", p=128))
    xres = singles.tile([128, 8, S_OWN], F32)      # residual stream (feature-major)
    e1t_sb = singles.tile([128, W], BF)
    e2t_sb = singles.tile([128, W], BF)
    m1bc_sb = singles.tile([128, W], BF)
    g3t_sb = singles.tile([128, LK], BF)
    plh1_sb = singles.tile([128, 2], BF)
    plh2_sb = singles.tile([128, 2], BF)

    def load_tables():
        # issued after the first projection so these ~4MB of attention-phase
        # tables don't sit ahead of the early weight loads in the sc queue
        sc.dma_start(xres, T["xow"].rearrange("(a p) r -> p a r", p=128))
        sc.dma_start(e1t_sb, T["e1t"])
        sc.dma_start(e2t_sb, T["e2t"])
        sc.dma_start(m1bc_sb, T["m1bc"])
        sc.dma_start(g3t_sb, T["g3t"])
        sc.dma_start(plh1_sb, T["plh1"])
        sc.dma_start(plh2_sb, T["plh2"])
    eps_sb = singles.tile([1, 1], F32)
    ve.memset(eps_sb, EPS)
    ones_sb = singles.tile([128, 1], BF)
    ve.memset(ones_sb, 1.0)
    enc = singles.tile([128, 8, LK], BF)           # current sublayer's enc input
    ktp = singles.tile([128, 8, LK], BF)           # K^T head pairs
    v_all = singles.tile([128, 8, H, 65], BF)      # V key-major [p, kt, h, dh+1]
    gp.memset(v_all, 1.0)
    qtp = singles.tile([128, 8, S_OWN], BF)        # Q^T head pairs
    aT = singles.tile([128, 8, S_OWN], BF)         # attention output (feature-major)
    h1 = singles.tile([128, 32, S_OWN], BF)        # FFN hidden
    ident = singles.tile([128, 128], BF)
    from concourse.masks import make_identity
    make_identity(nc, ident)

    def load_enc(name):
        sy.dma_start(enc, T[name].rearrange("(a p) r -> p a r", p=128))

    def bias_ap(col, base=0, size=128):
        return bias_sb[base:base + size, col:col + 1]

    def qh(h):
        """[64, S_OWN] view of head h's Q^T."""
        o = 64 * (h % 2)
        return qtp[o:o + 64, h // 2, :]

    def kh(h):
        o = 64 * (h % 2)
        return ktp[o:o + 64, h // 2, :]

    # ---------------- projections (lhsT-tiled weights) ----------------
    def proj(wname, nct, nkt, rhs_fn, nrc, epilogue, wtag):
        wd = T[wname]
        for ct in range(nct):
            wt = wpool.tile([128, nkt, 128], BF, tag=wtag,
                            bufs=(3 if wtag == "w8" else 2),
                            name=f"wt_{wname}_{ct}")
            (sy if ct % 2 == 0 else sc).dma_start(wt, wd[ct])
            pss = []
            for rc in range(nrc):
                ps = psum.tile([128, 512], F32, tag="mm", bufs=4,
                               name=f"ps_{wname}_{ct}_{rc}")
                pss.append(ps)
            for kt in range(nkt):
                for rc in range(nrc):
                    te.matmul(pss[rc], lhsT=wt[:, kt, :], rhs=rhs_fn(kt, rc),
                              start=(kt == 0), stop=(kt == nkt - 1))
            for rc in range(nrc):
                epilogue(ct, rc, pss[rc])

    def rhs_own(kt, rc):
        return xbf[:, kt, :]

    def rhs_enc(kt, rc):
        return enc[:, kt, 512 * rc:512 * rc + 512]

    def q_ep(bias0):
        def ep(ct, rc, ps):
            ve.tensor_scalar_add(qtp[:, ct, :], ps, bias_ap(bias0 + ct))
        return ep

    def k_ep(bias0, add_g):
        def ep(ct, rc, ps):
            dst = ktp[:, ct, 512 * rc:512 * rc + 512]
            b = bias_ap(bias0 + ct)
            if add_g:
                ve.scalar_tensor_tensor(
                    dst, ps, b, g3t_sb[:, 512 * rc:512 * rc + 512],
                    op0=AL.add, op1=AL.add)
            else:
                ve.tensor_scalar_add(dst, ps, b)
        return ep

    def v_proj(wname):
        """Key-major V: lhsT=enc tile, rhs=natural-layout Wv.  Bias folded
        into the O-projection bias host-side."""
        wd = T[wname]
        for c2 in range(2):
            wv = wpool.tile([128, 8, 512], BF, tag="w32", name=f"wv_{wname}_{c2}")
            (sy if c2 == 0 else sc).dma_start(wv, wd[:, :, 512 * c2:512 * c2 + 512])
            for kt in range(8):
                vps = psum.tile([128, 512], F32, tag="mm", bufs=4,
                                name=f"vps_{wname}_{c2}_{kt}")
                for fc in range(8):
                    te.matmul(vps, lhsT=enc[:, fc, 128 * kt:128 * kt + 128],
                              rhs=wv[:, fc, :],
                              start=(fc == 0), stop=(fc == 7))
                dst = v_all[:, kt, 8 * c2:8 * c2 + 8, 0:64]
                ve.tensor_copy(dst, vps.rearrange("p (a c) -> p a c", c=64))
        # ones column survives: only cols 0:64 of each head are written

    # ---------------- attention ----------------
    def attention(sl):
        e_sb = e1t_sb if sl == 1 else e2t_sb
        causal = (sl == 1)
        copy_flip = [0]

        def psum_to_sbuf(dst, src, add_mask_cols=None):
            """Alternate PSUM->SBUF copies between ve and sc."""
            eng = ve if (copy_flip[0] % 2 == 0) else None
            copy_flip[0] += 1
            if add_mask_cols is not None:
                ve.tensor_tensor(dst, src, add_mask_cols, op=AL.add)
            elif eng is ve:
                ve.tensor_copy(dst, src)
            else:
                sc.activation(dst, src, AF.Copy)

        plh_sb = plh1_sb if sl == 1 else plh2_sb

        def bands_for(h):
            """Q x E -> bf16 -> DRAM (trimmed col ranges) + clip biases."""
            bds = []
            blhs = {}
            o = 64 * (h % 2)
            for l in range(4):
                ranges = BAND_RANGES[(causal, l)]
                bsb = work.tile([128, W], BF, tag="bsb", bufs=2,
                                name=f"bsb{sl}_{h}_{l}")
                for (cs, wseg) in ranges:
                    bp = psum.tile([128, wseg], F32, tag="mm", bufs=4,
                                   name=f"bp{sl}_{h}_{l}_{cs}")
                    te.matmul(bp, lhsT=qh(h)[:, 128 * l:128 * l + 128],
                              rhs=e_sb[o:o + 64, cs:cs + wseg],
                              start=True, stop=True)
                    psum_to_sbuf(bsb[:, cs:cs + wseg], bp,
                                 add_mask_cols=(m1bc_sb[:, cs:cs + wseg]
                                                if causal else None))
                lo, hi = ranges[0][0], ranges[-1][0] + ranges[-1][1]
                bd = dram.tile([128, W], BF, tag="bd", bufs=12,
                               name=f"bd{sl}_{h}_{l}")
                sy.dma_start(bd[:, lo:hi], bsb[:, lo:hi])
                bds.append(bd)
                pass
            return bds, blhs

        def pv_head(h, ptb):
            pv = psum.tile([65, 512], F32, tag="pv", bufs=2, name=f"pv{sl}_{h}")
            if causal:
                for l in range(4):
                    kts = _sub1_kts(l)
                    for kt in kts:
                        te.matmul(pv[:, 128 * l:128 * l + 128],
                                  lhsT=v_all[:, kt, h, :],
                                  rhs=ptb[:, kt, 128 * l:128 * l + 128],
                                  start=(kt == kts[0]), stop=(kt == kts[-1]))
            else:
                for kt in range(8):
                    te.matmul(pv, lhsT=v_all[:, kt, h, :], rhs=ptb[:, kt, :],
                              start=(kt == 0), stop=(kt == 7))
            rzs = smalls.tile([1, 512], F32, tag="rz", bufs=2, name=f"rzs{sl}_{h}")
            sc.activation(rzs, pv[64:65, :], AF.Copy)
            rz = smalls.tile([1, 512], F32, tag="rz", bufs=2, name=f"rz{sl}_{h}")
            ve.reciprocal_approx_fast(rz, rzs)
            zb = smalls.tile([64, 512], F32, tag="zb", bufs=1, name=f"zb{sl}_{h}")
            gp.partition_broadcast(zb, rz)
            o = 64 * (h % 2)
            ve.tensor_mul(aT[o:o + 64, h // 2, :], pv[0:64, :], zb)
            if debug and sl == 1 and h == 0:
                sy.dma_start(T["d_rzs"], rzs)
                sy.dma_start(T["d_rz"], rz)

        ptb_prev = None
        bd_cur, blh_cur = bands_for(0) if sl != 3 else (None, None)
        for h in range(H):
            # issue next head's bands early so strip DMAs have lead time
            bd_next, blh_next = (bands_for(h + 1)
                                 if (sl != 3 and h + 1 < H) else (None, None))
            ptb = ppool.tile([128, 8, S_OWN], BF, tag="ptb", bufs=3,
                             name=f"ptb{sl}_{h}")
            pend = []

            def flush_one():
                pt0, kts0, l0 = pend.pop(0)
                tp = psum.tile([128, len(kts0), 128], BF, tag="tp", bufs=2,
                               name=f"tp{sl}_{h}_{l0}_{kts0[0]}")
                for ji in range(len(kts0)):
                    te.transpose(tp[:, ji, :], pt0[:, 128 * ji:128 * ji + 128],
                                 ident)
                dst = ptb[:, kts0[0]:kts0[0] + len(kts0),
                          128 * l0:128 * l0 + 128]
                if (l0 + kts0[0]) % 2 == 0:
                    ve.tensor_copy(dst, tp)
                else:
                    sc.activation(dst, tp, AF.Copy)

            for l in range(4):
                kt_list = _sub1_kts(l) if causal else list(range(8))
                bd = bd_cur[l] if sl != 3 else None
                # --- scores + softmax chunks of <=4 kts ---
                for c0 in range(0, len(kt_list), 4):
                    kts = kt_list[c0:c0 + 4]
                    n = 128 * len(kts)
                    sps = psum.tile([128, n], F32, tag="mm", bufs=4,
                                    name=f"s{sl}_{h}_{l}_{c0}")
                    te.matmul(sps, lhsT=qh(h)[:, 128 * l:128 * l + 128],
                              rhs=kh(h)[:, 128 * kts[0]:128 * kts[0] + n],
                              start=True, stop=True)
                    pt = ppool.tile([128, n], BF, tag="p", bufs=4,
                                    name=f"p{sl}_{h}_{l}_{c0}")
                    ctype = "none" if sl == 3 else "strip"
                    if sl == 3:
                        sc.activation(pt, sps, AF.Exp, scale=1.0 / SCALE)
                    elif ctype != "strip":
                        col = 0 if ctype == "blo" else 1
                        sc.activation(pt, sps, AF.Exp, scale=1.0 / SCALE,
                                      bias=blh_cur[l][:, col:col + 1])
                    else:
                        strip = work.tile([128, n], BF, tag="strip", bufs=2,
                                          name=f"strip{sl}_{h}_{l}_{c0}")
                        for (dcol, nseg, t0, affine) in _skew_segments(l, kts):
                            if affine:
                                src = bass.AP(tensor=bd.tensor,
                                              offset=bd.offset + t0,
                                              ap=[[W - 1, 128], [1, 128 * nseg]])
                                sy.dma_start(strip[:, dcol:dcol + 128 * nseg], src)
                            else:
                                src = bass.AP(tensor=bd.tensor,
                                              offset=bd.offset + t0,
                                              ap=[[W - 1, 128], [0, nseg], [1, 128]])
                                dst = strip[:, dcol:dcol + 128 * nseg].rearrange(
                                    "p (n w) -> p n w", w=128)
                                sy.dma_start(dst, src)
                        lg = work.tile([128, n], F32, tag="lg", bufs=3,
                                       name=f"lg{sl}_{h}_{l}_{c0}")
                        ve.scalar_tensor_tensor(lg, sps, 1.0 / SCALE, strip,
                                                op0=AL.mult, op1=AL.add)
                        sc.activation(pt, lg, AF.Exp)
                        if debug and sl == 1 and h == 0 and l == 3 and c0 == 0:
                            sy.dma_start(T["d_strip"], strip)
                            sy.dma_start(T["d_lg"], lg)
                            sy.dma_start(T["d_p"], pt)
                    # --- P^T transposes run 2 chunks behind so the
                    # stt->exp chain hides under later scores ---
                    pend.append((pt, kts, l))
                    if len(pend) > 2:
                        flush_one()
            # --- PV for the PREVIOUS head (its ptb completed during this
            # head's scores, so the PE never waits on the exp chain) ---
            if h >= 1:
                pv_head(h - 1, ptb_prev)
            while pend:
                flush_one()
            ptb_prev = ptb
            bd_cur, blh_cur = bd_next, blh_next
        pv_head(H - 1, ptb_prev)

    # ---------------- output proj + residual ----------------
    def o_proj(wname, bias0):
        def ep(ct, rc, ps):
            ve.scalar_tensor_tensor(xres[:, ct, :], ps, bias_ap(bias0 + ct),
                                    xres[:, ct, :], op0=AL.add, op1=AL.add)
        proj(wname, 8, 8, lambda kt, rc: aT[:, kt, :], 1, ep, "w8")

    # ---------------- layernorm (feature-major) ----------------
    def layer_norm(gname, bname, final_out=None):
        sfx = 1 if final_out is None else 2
        s1 = psum.tile([1, 512], F32, tag="pv", bufs=2, name=f"lns1_{gname}_{sfx}")
        s2 = psum.tile([1, 512], F32, tag="pv", bufs=2, name=f"lns2_{gname}_{sfx}")
        for dt in range(8):
            bx = work.tile([128, 512], BF, tag="lnbx", bufs=2, name=f"lnbx_{gname}_{dt}")
            sc.activation(bx, xres[:, dt, :], AF.Copy)
            sq = work.tile([128, 512], BF, tag="lnsq", bufs=2, name=f"lnsq_{gname}_{dt}")
            ve.tensor_mul(sq, xres[:, dt, :], xres[:, dt, :])
            te.matmul(s1, lhsT=ones_sb, rhs=bx,
                      start=(dt == 0), stop=(dt == 7))
            te.matmul(s2, lhsT=ones_sb, rhs=sq,
                      start=(dt == 0), stop=(dt == 7))
        mean = smalls.tile([1, 512], F32, tag="rz", bufs=2, name=f"lnmean_{gname}")
        ve.tensor_scalar_mul(mean, s1, 1.0 / D)
        rstd = smalls.tile([1, 512], F32, tag="rz", bufs=2, name=f"lnrstd_{gname}")
        ve.tensor_mul(rstd, mean, mean)                                  # mean^2
        ve.scalar_tensor_tensor(rstd, s2, 1.0 / D, rstd,
                                op0=AL.mult, op1=AL.subtract)            # var
        sc.activation(rstd, rstd, AF.Sqrt, bias=eps_sb)                  # sd
        ve.reciprocal_approx_fast(rstd, rstd)                            # 1/sd
        mb = work.tile([128, 512], F32, tag="lnb", bufs=2, name=f"lnmb_{gname}")
        gp.partition_broadcast(mb, mean)
        rb = work.tile([128, 512], F32, tag="lnb", bufs=2, name=f"lnrb_{gname}")
        gp.partition_broadcast(rb, rstd)
        gcol, bcol = BIAS_COL[gname], BIAS_COL[bname]
        for dt in range(8):
            mgr = work.tile([128, 512], F32, tag="lg", bufs=3, name=f"lnmgr_{gname}_{dt}")
            ve.scalar_tensor_tensor(mgr, mb, bias_ap(gcol + dt), rb,
                                    op0=AL.mult, op1=AL.mult)
            cc = work.tile([128, 512], F32, tag="lg", bufs=3, name=f"lncc_{gname}_{dt}")
            ve.tensor_scalar(cc, mgr, -1.0, bias_ap(bcol + dt),
                             op0=AL.mult, op1=AL.add)
            t = work.tile([128, 512], F32, tag="lg", bufs=3, name=f"lnt_{gname}_{dt}")
            ve.scalar_tensor_tensor(t, xres[:, dt, :], bias_ap(gcol + dt), rb,
                                    op0=AL.mult, op1=AL.mult)
            if final_out is not None:
                ot = work.tile([128, 512], F32, tag="lg", bufs=3, name=f"lnot_{gname}_{dt}")
                ve.tensor_add(ot, t, cc)
                sy.dma_start(final_out[128 * dt:128 * dt + 128, :], ot)
            else:
                ve.tensor_add(xres[:, dt, :], t, cc)
                sc.activation(xbf[:, dt, :], xres[:, dt, :], AF.Copy)

    # ================= sublayer 1 =================
    load_enc("sfb")
    proj("wq1", 8, 8, rhs_own, 1, q_ep(BIAS_COL["q1"]), "w8")
    load_tables()
    proj("wk1", 8, 8, rhs_enc, 2, k_ep(BIAS_COL["k1"], False), "w8")
    v_proj("wv1")
    load_enc("chb")          # prefetch sublayer-2 encoder during attention 1
    if debug:
        sy.dma_start(T["d_qt"], qtp)
        sy.dma_start(T["d_kt"], ktp)
        sy.dma_start(T["d_v"], v_all[:, :, 0, :])
    attention(1)
    if debug:
        sy.dma_start(T["d_at"], aT)
    o_proj("wo1", BIAS_COL["o1"])
    layer_norm("ln1g", "ln1b")
    if debug:
        sy.dma_start(T["d_x1"], xres)

    # ================= sublayer 2 =================
    proj("wk2", 8, 8, rhs_enc, 2, k_ep(BIAS_COL["k2"], False), "w8")
    v_proj("wv2")
    load_enc("wdb")          # prefetch sublayer-3 encoder during attention 2
    proj("wq2", 8, 8, rhs_own, 1, q_ep(BIAS_COL["q2"]), "w8")
    attention(2)
    o_proj("wo2", BIAS_COL["o2"])
    layer_norm("ln2g", "ln2b")

    # ================= sublayer 3 =================
    proj("wk3", 8, 8, rhs_enc, 2, k_ep(BIAS_COL["k3"], True), "w8")
    v_proj("wv3")
    proj("wq3", 8, 8, rhs_own, 1, q_ep(BIAS_COL["q3"]), "w8")
    attention(3)
    o_proj("wo3", BIAS_COL["o3"])
    layer_norm("ln3g", "ln3b")

    # ================= FFN =================
    def f1_ep(ct, rc, ps):
        sc.activation(h1[:, ct, :], ps, AF.Relu, bias=bias_ap(BIAS_COL["f1"] + ct))
    proj("wf1", 32, 8, rhs_own, 1, f1_ep, "w8")

    def f2_ep(ct, rc, ps):
        ve.scalar_tensor_tensor(xres[:, ct, :], ps, bias_ap(BIAS_COL["f2"] + ct),
                                xres[:, ct, :], op0=AL.add, op1=AL.add)
    proj("wf2", 8, 32, lambda kt, rc: h1[:, kt, :], 1, f2_ep, "w32")

    layer_norm("ln3g", "ln3b", final_out=T["yT"])


def build_nc(debug=False):
    nc = bacc.Bacc("TRN2", target_bir_lowering=False, debug=False)
    T = {}

    def din(name, shape, dt=BF):
        T[name] = nc.dram_tensor(name, shape, dt, kind="ExternalInput").ap()

    din("xow", [D, S_OWN], F32)
    din("xob", [D, S_OWN])
    din("sfb", [D, LK])
    din("chb", [D, LK])
    din("wdb", [D, LK])
    for w in ["wq1", "wk1", "wq2", "wk2", "wq3", "wk3", "wo1", "wo2", "wo3"]:
        din(w, [8, 128, 8, 128])
    for w in ["wv1", "wv2", "wv3"]:
        din(w, [128, 8, 1024])
    din("wf1", [32, 128, 8, 128])
    din("wf2", [8, 128, 32, 128])
    din("bias", [128, NBIAS], F32)
    din("e1t", [128, W])
    din("e2t", [128, W])
    din("m1bc", [128, W])
    din("plh1", [128, 2])
    din("plh2", [128, 2])
    din("g3t", [128, LK])
    T["yT"] = nc.dram_tensor("yT", [D, S_OWN], F32, kind="ExternalOutput").ap()
    if debug:
        def dout(name, shape, dt=BF):
            T[name] = nc.dram_tensor(name, shape, dt, kind="ExternalOutput").ap()
        dout("d_qt", [128, 8, S_OWN])
        dout("d_kt", [128, 8, LK])
        dout("d_v", [128, 8, 65])
        dout("d_at", [128, 8, S_OWN])
        dout("d_x1", [128, 8, S_OWN], F32)
        dout("d_band", [128, W])
        dout("d_strip", [128, 512])
        dout("d_lg", [128, 512], F32)
        dout("d_p", [128, 512])
        dout("d_rzs", [1, 512], F32)
        dout("d_rz", [1, 512], F32)

    from contextlib import ExitStack
    with tile.TileContext(nc) as tc:
        with ExitStack() as ctx:
            _emit(nc, tc, ctx, T, debug=debug)
    nc.compile()
    return nc


_NC = None


def _get_nc():
    global _NC
    if _NC is None:
        _NC = build_nc()
    return _NC


# ======================= host side =======================

def _own_rows(pi):
    return np.concatenate([np.arange(128 * (2 * l + pi), 128 * (2 * l + pi) + 128)
                           for l in range(4)])


def _tile_lhsT(w):
    """[K, N] -> [nct, 128, nkt, 128]: t[ct, p, kt, c] = w[kt*128+p, ct*128+c]."""
    K, N = w.shape
    return np.ascontiguousarray(
        w.reshape(K // 128, 128, N // 128, 128).transpose(2, 1, 0, 3)
    ).astype(BF16)


def _wv_layout(w):
    """[K, N] -> [128, 8, N]: t[p, fc, d] = w[fc*128+p, d]."""
    K, N = w.shape
    return np.ascontiguousarray(
        w.reshape(K // 128, 128, N).transpose(1, 0, 2)
    ).astype(BF16)


def _stripe(v):
    """bias vector [n*128] -> [128, n] per-partition stripes (fp32)."""
    n = v.shape[0] // 128
    return np.ascontiguousarray(v.reshape(n, 128).T).astype(np.float32)


def _build_E(pos_scaled, pi):
    C = TB + 128 * pi
    idx = np.clip(np.arange(W) - C, -M, M) + M
    e = np.ascontiguousarray(pos_scaled[idx].T).astype(BF16)  # [64, W]
    return np.concatenate([e, e], axis=0)  # both partition halves


def _build_mask(pi):
    C = TB + 128 * pi
    row = np.where(np.arange(W) - C > 0, NEG, 0.0).astype(np.float32)
    return np.broadcast_to(row, (128, W)).astype(BF16)


def _qpos(sentence_lengths):
    s = np.asarray(sentence_lengths, np.int64)
    offsets = s - np.cumsum(s)
    B = int(s.sum())
    return np.repeat(offsets, s)[:B] + np.arange(B)


def _host_prep(inp):
    qkv_w = np.asarray(inp["qkv_w"], np.float32)
    wq = qkv_w.reshape(D, H, 3, DH)
    q1w = wq[:, :, 0].reshape(D, D)
    k1w = wq[:, :, 1].reshape(D, D)
    v1w = wq[:, :, 2].reshape(D, D)
    qb = np.asarray(inp["qkv_b"], np.float32).reshape(H, 3, DH)
    q1b, k1b, v1b = qb[:, 0].reshape(D), qb[:, 1].reshape(D), qb[:, 2].reshape(D)

    # fold V biases into O biases: (a + bv) @ Wo + bo == a @ Wo + (bo + bv @ Wo)
    o1w = np.asarray(inp["o1_w"], np.float32)
    o2w = np.asarray(inp["o2_w"], np.float32)
    o3w = np.asarray(inp["o3_w"], np.float32)
    o1b = np.asarray(inp["o1_b"], np.float32) + v1b @ o1w
    o2b = (np.asarray(inp["o2_b"], np.float32)
           + np.asarray(inp["v2_b"], np.float32) @ o2w)
    o3b = (np.asarray(inp["o3_b"], np.float32)
           + np.asarray(inp["v3_b"], np.float32) @ o3w)

    bias = np.zeros((128, NBIAS), np.float32)

    def put(name, vec):
        c = BIAS_COL[name]
        s = _stripe(np.asarray(vec, np.float32))
        bias[:, c:c + s.shape[1]] = s

    put("q1", q1b)
    put("k1", k1b)
    put("o1", o1b)
    put("o2", o2b)
    put("o3", o3b)
    for n, k in [("q2", "q2_b"), ("k2", "k2_b"), ("q3", "q3_b"), ("k3", "k3_b"),
                 ("f1", "f1_b"), ("f2", "f2_b"),
                 ("ln1g", "ln1_g"), ("ln1b", "ln1_b"), ("ln2g", "ln2_g"),
                 ("ln2b", "ln2_b"), ("ln3g", "ln3_g"), ("ln3b", "ln3_b")]:
        put(n, inp[k])

    weights = {
        "wq1": _tile_lhsT(q1w), "wk1": _tile_lhsT(k1w), "wv1": _wv_layout(v1w),
        "wo1": _tile_lhsT(o1w),
        "wq2": _tile_lhsT(np.asarray(inp["q2_w"], np.float32)),
        "wk2": _tile_lhsT(np.asarray(inp["k2_w"], np.float32)),
        "wv2": _wv_layout(np.asarray(inp["v2_w"], np.float32)),
        "wo2": _tile_lhsT(o2w),
        "wq3": _tile_lhsT(np.asarray(inp["q3_w"], np.float32)),
        "wk3": _tile_lhsT(np.asarray(inp["k3_w"], np.float32)),
        "wv3": _wv_layout(np.asarray(inp["v3_w"], np.float32)),
        "wo3": _tile_lhsT(o3w),
        "wf1": _tile_lhsT(np.asarray(inp["f1_w"], np.float32)),
        "wf2": _tile_lhsT(np.asarray(inp["f2_w"], np.float32)),
        "bias": bias,
    }

    pos1s = np.asarray(inp["pos1"], np.float32) / SCALE
    pos2s = np.asarray(inp["pos2"], np.float32) / SCALE
    pos3 = np.asarray(inp["pos3"], np.float32)
    def _plh(pos_scaled):
        p = np.stack([pos_scaled[0], pos_scaled[2 * M]], axis=1)  # [64, 2]
        return np.concatenate([p, p], axis=0).astype(BF16)        # [128, 2]

    plh1 = _plh(pos1s)
    plh2 = _plh(pos2s)
    e1 = [_build_E(pos1s, pi) for pi in range(2)]
    e2 = [_build_E(pos2s, pi) for pi in range(2)]
    m1 = [_build_mask(pi) for pi in range(2)]

    qpos = _qpos(inp["sentence_lengths"])
    g3 = []
    for b in range(4):
        idx = np.clip(np.arange(LK) - int(qpos[b]), -M, M) + M
        g = pos3[idx].T.astype(BF16)          # [64, LK]
        g3.append(np.concatenate([g, g], axis=0))  # [128, LK] duplicated

    x = np.asarray(inp["self_input"], np.float32)
    ch = np.asarray(inp["char_enc"], np.float32)
    wd = np.asarray(inp["word_enc"], np.float32)

    in_maps = []
    for core in range(8):
        b, pi = core // 2, core % 2
        rows = _own_rows(pi)
        xT = np.ascontiguousarray(x[b].T)            # [D, 1024]
        m = dict(weights)
        m["xow"] = np.ascontiguousarray(xT[:, rows])
        m["xob"] = m["xow"].astype(BF16)
        m["sfb"] = xT.astype(BF16)
        m["chb"] = np.ascontiguousarray(ch[b].T).astype(BF16)
        m["wdb"] = np.ascontiguousarray(wd[b].T).astype(BF16)
        m["e1t"] = e1[pi]
        m["e2t"] = e2[pi]
        m["m1bc"] = m1[pi]
        m["plh1"] = plh1
        m["plh2"] = plh2
        m["g3t"] = g3[b]
        in_maps.append(m)
    return in_maps


def _fast_path_ok(inp):
    lam = np.asarray(inp["look_ahead_mask"])
    B, Lq = 4, 1024
    if lam.shape != (1, 1, Lq, Lq):
        return False
    causal = np.triu(np.ones((Lq, Lq), bool), k=1)
    if not np.array_equal(lam[0, 0].astype(bool), causal):
        return False
    if np.asarray(inp["char_mask"]).any() or np.asarray(inp["word_mask"]).any():
        return False
    if np.asarray(inp["sentence_lengths"]).sum() != B:
        return False
    return True


def _numpy_reference(inp):
    """Pure-numpy fallback (slow but exact) for unexpected mask patterns."""
    f = lambda k: np.asarray(inp[k], np.float32)

    def ln(x, g, b):
        m = x.mean(-1, keepdims=True)
        v = ((x - m) ** 2).mean(-1, keepdims=True)
        return (x - m) / np.sqrt(v + EPS) * g + b

    def split_heads(x):
        B, S, _ = x.shape
        return x.reshape(B, S, H, DH).transpose(0, 2, 1, 3)

    def softmax(x):
        x = x - x.max(-1, keepdims=True)
        e = np.exp(x)
        return e / e.sum(-1, keepdims=True)

    def attn(Q, K, V, pl, mask):
        logits = (np.einsum('bhid,bhjd->bhij', Q, K) + pl) / SCALE
        logits = np.where(mask, -np.inf, logits)
        p = softmax(logits)
        out = np.einsum('bhij,bhjd->bhid', p, V)
        B, h, S, dh = out.shape
        return out.transpose(0, 2, 1, 3).reshape(B, S, h * dh)

    def char_pos(emb, lq, lk):
        idx = np.clip(np.arange(lk)[None, :] - np.arange(lq)[:, None], -M, M) + M
        return emb[idx]

    x0 = f("self_input")
    B, Lq, _ = x0.shape
    qkv = (x0 @ f("qkv_w") + f("qkv_b")).reshape(B, Lq, H, 3 * DH).transpose(0, 2, 1, 3)
    Q, K, V = np.split(qkv, 3, axis=-1)
    pl = np.einsum('bhid,ijd->bhij', Q, char_pos(f("pos1"), Lq, Lq))
    a = attn(Q, K, V, pl, np.asarray(inp["look_ahead_mask"])) @ f("o1_w") + f("o1_b")
    x = ln(a + x0, f("ln1_g"), f("ln1_b"))

    ce = f("char_enc")
    Q = split_heads(x @ f("q2_w") + f("q2_b"))
    K = split_heads(ce @ f("k2_w") + f("k2_b"))
    V = split_heads(ce @ f("v2_w") + f("v2_b"))
    pl = np.einsum('bhid,ijd->bhij', Q, char_pos(f("pos2"), Lq, ce.shape[1]))
    a = attn(Q, K, V, pl, np.asarray(inp["char_mask"])) @ f("o2_w") + f("o2_b")
    x = ln(a + x, f("ln2_g"), f("ln2_b"))

    we = f("word_enc")
    Q = split_heads(x @ f("q3_w") + f("q3_b"))
    K = split_heads(we @ f("k3_w") + f("k3_b"))
    V = split_heads(we @ f("v3_w") + f("v3_b"))
    qpos = _qpos(inp["sentence_lengths"])
    idx = np.clip(np.arange(we.shape[1])[None, :] - qpos[:, None], -M, M) + M
    pl = np.einsum('bhid,bjd->bhij', Q, f("pos3")[idx])
    a = attn(Q, K, V, pl, np.asarray(inp["word_mask"])) @ f("o3_w") + f("o3_b")
    x = ln(a + x, f("ln3_g"), f("ln3_b"))

    ffn = np.maximum(x @ f("f1_w") + f("f1_b"), 0.0) @ f("f2_w") + f("f2_b")
    return ln(ffn + x, f("ln3_g"), f("ln3_b"))


def kernel(**inputs) -> np.ndarray:
    if not _fast_path_ok(inputs):
        return _numpy_reference(inputs)
    nc = _get_nc()
    in_maps = _host_prep(inputs)
    res = run_bass_kernel_spmd(nc, in_maps, list(range(8)))
    y = np.empty((4, 1024, 1024), np.float32)
    for core in range(8):
        b, pi = core // 2, core % 2
        yT = res.results[core]["yT"]
        y[b, _own_rows(pi), :] = yT.T
    return y
